# revision 1
# baseline (speedup 1.0000x reference)
"""Trainium2 Bass kernel for ClassLinearWithLORA (moe_routing).

Computes out = x @ W.T + b + gates[-1] * (alpha * (x @ A[-1]) @ B_lora[-1])
(the torch loop overwrites out_lora each class iteration, so only the last
class adapter contributes).

Strategy:
  - Data-parallel shard of the 8192 (B*S) rows across 8 NeuronCores
    (1024 rows/core); W/b and the rank-16 LoRA stacks are replicated.
  - Matmuls run as fp32r (fp32 with mantissa rounded to 11 bits, TF32-like):
    1 cycle/row on the PE at N>=256 vs 4 cycles/row for full fp32.
    Inputs are pre-rounded on the host so DMAs are pure copies.
  - Formulation: psum[r128, o512] = sum_k xT[k][:, r].T @ WT[k][:, o]
    accumulated over 8 K-tiles, plus ONE augmented K=17 matmul that adds
    both the LoRA rank-16 update and the bias:
       lhsT_aug = [ (g * (x @ A).T) ; ones ]  (17 x r)
       rhs_aug  = [ alpha * B_lora[-1] ; b ]  (17 x o)
    The gate is folded into the rank-16 intermediate (per-row scale
    commutes with the second LoRA matmul); alpha is folded into B.
  - PSUM->SBUF copies and the gate multiply run on the Vector engine.
    Weight blocks + small tensors stream on the SP HWDGE ring while xt
    chunks stream in parallel on the ACT ring (which later carries the
    output stores); the ob=0 row tiles are emitted interleaved with the
    LoRA matmuls per K-chunk so the PE static order is paced by xt-chunk
    arrivals instead of stalling on the full resident load.

Measured (8 cores, full inputs): relative error 1.7e-4 vs the fp32 jax
reference; per-core cost-model time 144.0 us (PE-bound; fp32r matmul
roofline for this decomposition is ~126 us/core).
"""

import numpy as np

import concourse.bacc as bacc
import concourse.mybir as mybir
import concourse.tile as tile
from concourse.bass_utils import run_bass_kernel_spmd

F32 = mybir.dt.float32
F32R = mybir.dt.float32r

N_CORES = 8
B, S, D_IN, D_OUT, R_LORA = 4, 2048, 1024, 4096, 16
ROWS = B * S                  # 8192
R_CORE = ROWS // N_CORES      # 1024 rows per core
KT = D_IN // 128              # 8 K-tiles of 128
NB = 512                      # moving free dim per matmul (max for 4-byte)
OB = D_OUT // NB              # 8 output blocks
RT = R_CORE // 128            # 8 row tiles per core
KA = R_LORA + 1               # augmented contraction (16 LoRA + 1 bias)


def _round_fp32r(a: np.ndarray) -> np.ndarray:
    """Round fp32 to the fp32r-representable set (11-bit mantissa,
    round-half-up in magnitude, carry into exponent OK)."""
    a = np.ascontiguousarray(a, dtype=np.float32)
    u = a.view(np.uint32)
    r = ((u + np.uint32(0x800)) & np.uint32(0xFFFFF000)).astype(np.uint32)
    return r.view(np.float32)


AUG_FIRST = True

def _build(
    xt_chunks: int = 8,
    xt_engine: str = "scalar",
    wt_bufs: int = 3,
    psum_bufs: int = 8,
    out_bufs: int = 4,
    wt0_split: int = 8,
    wt_split: int = 4,
    wt_alternate: bool = False,
):
    nc = bacc.Bacc(None, target_bir_lowering=False)

    x_d = nc.dram_tensor("xt", [128, KT, R_CORE], F32R, kind="ExternalInput")
    w_d = nc.dram_tensor("wt", [128, OB, KT, NB], F32R, kind="ExternalInput")
    a_d = nc.dram_tensor("a_lora", [128, KT, R_LORA], F32R, kind="ExternalInput")
    rhs_d = nc.dram_tensor("aug_rhs", [KA, D_OUT], F32R, kind="ExternalInput")
    g_d = nc.dram_tensor("g_rep", [R_LORA, R_CORE], F32, kind="ExternalInput")
    one_d = nc.dram_tensor("ones_row", [1, R_CORE], F32R, kind="ExternalInput")
    out_d = nc.dram_tensor("out", [R_CORE, D_OUT], F32, kind="ExternalOutput")

    with tile.TileContext(nc) as tc:
        with (
            tc.tile_pool(name="resident", bufs=1) as res,
            tc.tile_pool(name="wpool", bufs=wt_bufs) as wpool,
            tc.tile_pool(name="opool", bufs=out_bufs) as opool,
            tc.tile_pool(name="psum", bufs=psum_bufs, space="PSUM") as psum,
        ):
            # ---- resident loads -------------------------------------------------
            # Critical path at t=0 is (small tensors) + (wt block 0) on the SP
            # ring and xt chunk k on the ACT ring. wt block 0 is split along K
            # so the first matmuls unblock early.
            ld = getattr(nc, xt_engine)
            a_sb = res.tile([128, KT, R_LORA], F32R)
            nc.sync.dma_start(a_sb[:], a_d.ap())
            wt0 = wpool.tile([128, KT, NB], F32R, tag="wt")
            kh = KT // wt0_split
            for h in range(wt0_split):
                nc.sync.dma_start(
                    wt0[:, h * kh : (h + 1) * kh, :],
                    w_d.ap()[:, 0, h * kh : (h + 1) * kh, :],
                )
            # g/rhs/ones are not consumed until the gate multiply and first
            # aug matmul (~18us in) — load them after wt block 0
            g_sb = res.tile([R_LORA, R_CORE], F32)
            nc.sync.dma_start(g_sb[:], g_d.ap())
            rhs_sb = res.tile([KA, D_OUT], F32R)
            nc.sync.dma_start(rhs_sb[:], rhs_d.ap())
            lora_aug = res.tile([KA, R_CORE], F32R)
            nc.sync.dma_start(lora_aug[R_LORA : R_LORA + 1, :], one_d.ap())
            xt = res.tile([128, KT, R_CORE], F32R)
            if xt_chunks <= KT:
                kc = KT // xt_chunks
                for k in range(xt_chunks):
                    if k == 0 and kc == 1:
                        # split the first chunk in R-halves: the first lora +
                        # main matmuls unblock after 256KB instead of 512KB
                        hr = R_CORE // 2
                        ld.dma_start(xt[:, 0, 0:hr], x_d.ap()[:, 0, 0:hr])
                        ld.dma_start(xt[:, 0, hr:R_CORE], x_d.ap()[:, 0, hr:R_CORE])
                        continue
                    ld.dma_start(
                        xt[:, k * kc : (k + 1) * kc, :],
                        x_d.ap()[:, k * kc : (k + 1) * kc, :],
                    )
            else:
                rsplit = xt_chunks // KT
                rc = R_CORE // rsplit
                for k in range(KT):
                    for rh in range(rsplit):
                        ld.dma_start(
                            xt[:, k, rh * rc : (rh + 1) * rc],
                            x_d.ap()[:, k, rh * rc : (rh + 1) * rc],
                        )
            def emit_epilogue(ps, rt, ob):
                """Close psum tile: copy to SBUF, then store. For the last
                o-block, split copy+store in halves across both HWDGE rings
                (the SP ring is load-free by then) to shorten the tail chain."""
                o_sb = opool.tile([128, NB], F32, tag="o_sb", name=f"o_{ob}_{rt}")
                orow = out_d.ap()[rt * 128 : (rt + 1) * 128, ob * NB : (ob + 1) * NB]
                if ob == OB - 1:
                    h = NB // 2
                    nc.vector.tensor_copy(o_sb[:, 0:h], ps[:, 0:h])
                    nc.scalar.dma_start(orow[:, 0:h], o_sb[:, 0:h])
                    nc.vector.tensor_copy(o_sb[:, h:NB], ps[:, h:NB])
                    nc.sync.dma_start(orow[:, h:NB], o_sb[:, h:NB])
                else:
                    nc.vector.tensor_copy(o_sb[:], ps[:])
                    nc.scalar.dma_start(orow[:], o_sb[:])

            def emit_aug(ps, rt, ob, start, stop):
                # rank-16 LoRA update + bias in one K=17 matmul
                nc.tensor.matmul(
                    ps[:],
                    lora_aug[:, rt * 128 : (rt + 1) * 128],
                    rhs_sb[:, ob * NB : (ob + 1) * NB],
                    start=start,
                    stop=stop,
                )

            # ---- prologue: ob=0 interleaved with the LoRA first matmul ---------
            # PE static order is paced by xt-chunk arrivals, so per K-chunk we
            # emit the 2 lora matmuls plus 6 of the 8 ob=0 row tiles (2 lora +
            # 6 main psum tiles = 8 banks). rt=6,7 run densely afterwards.
            NRB = R_CORE // NB  # lora row blocks
            ps_l = [psum.tile([R_LORA, NB], F32, tag="ps", name=f"psl{rb}") for rb in range(NRB)]
            ps0 = [psum.tile([128, NB], F32, tag="ps", name=f"ps0_{rt}") for rt in range(6)]
            for k in range(KT):
                for rb in range(NRB):
                    nc.tensor.matmul(
                        ps_l[rb][:],
                        a_sb[:, k, :],
                        xt[:, k, rb * NB : (rb + 1) * NB],
                        start=(k == 0),
                        stop=(k == KT - 1),
                    )
                for rt in range(6):
                    nc.tensor.matmul(
                        ps0[rt][:],
                        xt[:, k, rt * 128 : (rt + 1) * 128],
                        wt0[:, k, :],
                        start=(k == 0),
                        stop=False,
                    )
            # gate multiply, rounded to fp32r for the augmented matmul
            for rb in range(NRB):
                nc.vector.tensor_mul(
                    lora_aug[0:R_LORA, rb * NB : (rb + 1) * NB],
                    ps_l[rb][:],
                    g_sb[:, rb * NB : (rb + 1) * NB],
                )
            for rt in range(6):
                emit_aug(ps0[rt], rt, 0, start=False, stop=True)
                emit_epilogue(ps0[rt], rt, 0)
            for rt in (6, 7):
                ps = psum.tile([128, NB], F32, tag="ps", name=f"ps0b_{rt}")
                for k in range(KT):
                    nc.tensor.matmul(
                        ps[:],
                        xt[:, k, rt * 128 : (rt + 1) * 128],
                        wt0[:, k, :],
                        start=(k == 0),
                        stop=False,
                    )
                emit_aug(ps, rt, 0, start=False, stop=True)
                emit_epilogue(ps, rt, 0)

            # ---- steady state: ob = 1..7 ---------------------------------------
            for ob in range(1, OB):
                wt = wpool.tile([128, KT, NB], F32R, tag="wt", name=f"wt{ob}")
                kw = KT // wt_split
                # alternate rings: the ACT ring is load-free once xt lands,
                # doubling weight delivery rate while the pipeline catches up
                wt_eng = nc.scalar if (ob % 2 == 1 and wt_alternate) else nc.sync
                for h in range(wt_split):
                    wt_eng.dma_start(
                        wt[:, h * kw : (h + 1) * kw, :],
                        w_d.ap()[:, ob, h * kw : (h + 1) * kw, :],
                    )
                for rt in range(RT):
                    ps = psum.tile([128, NB], F32, tag="ps", name=f"ps{ob}_{rt}")
                    if AUG_FIRST:
                        emit_aug(ps, rt, ob, start=True, stop=False)
                    for k in range(KT):
                        nc.tensor.matmul(
                            ps[:],
                            xt[:, k, rt * 128 : (rt + 1) * 128],
                            wt[:, k, :],
                            start=(not AUG_FIRST and k == 0),
                            stop=(AUG_FIRST and k == KT - 1),
                        )
                    if not AUG_FIRST:
                        emit_aug(ps, rt, ob, start=False, stop=True)
                    emit_epilogue(ps, rt, ob)

    nc.compile()
    return nc


_NC_CACHE = None


def _get_nc():
    global _NC_CACHE
    if _NC_CACHE is None:
        _NC_CACHE = _build()
    return _NC_CACHE


def _prep_in_maps(x, W, b, A, B_lora, gates, alpha):
    x = np.asarray(x, dtype=np.float32).reshape(ROWS, D_IN)
    W = np.asarray(W, dtype=np.float32)
    b = np.asarray(b, dtype=np.float32)
    A_last = np.asarray(A, dtype=np.float32)[-1]          # [D_IN, 16]
    B_last = np.asarray(B_lora, dtype=np.float32)[-1]     # [16, D_OUT]
    g_last = np.asarray(gates, dtype=np.float32)[-1].reshape(ROWS)
    alpha_f = float(np.asarray(alpha))

    # W.T packed as [ki, ob, ko, o'] so each o-block DMA is one contiguous
    # 16 KiB run per partition.
    wt = W.T.reshape(KT, 128, OB, NB).transpose(1, 2, 0, 3)
    w_pre = _round_fp32r(np.ascontiguousarray(wt))

    a_pre = _round_fp32r(
        np.ascontiguousarray(A_last.reshape(KT, 128, R_LORA).transpose(1, 0, 2))
    )
    aug = np.concatenate([alpha_f * B_last, b[None, :]], axis=0)  # [17, D_OUT]
    aug_pre = _round_fp32r(aug)
    ones_row = _round_fp32r(np.ones((1, R_CORE), dtype=np.float32))

    in_maps = []
    for c in range(N_CORES):
        rows = slice(c * R_CORE, (c + 1) * R_CORE)
        xs = x[rows]                                      # [R_CORE, D_IN]
        xt = xs.T.reshape(KT, 128, R_CORE).transpose(1, 0, 2)
        x_pre = _round_fp32r(np.ascontiguousarray(xt))
        g_rep = np.ascontiguousarray(
            np.broadcast_to(g_last[rows][None, :], (R_LORA, R_CORE))
        ).astype(np.float32)
        in_maps.append(
            {
                "xt": x_pre,
                "wt": w_pre,
                "a_lora": a_pre,
                "aug_rhs": aug_pre,
                "g_rep": g_rep,
                "ones_row": ones_row,
            }
        )
    return in_maps


def run(inputs: dict, trace: bool = False, trace_cores=None):
    """Run the kernel; returns (full_output, BassKernelResults)."""
    nc = _get_nc()
    in_maps = _prep_in_maps(**inputs)
    res = run_bass_kernel_spmd(
        nc,
        in_maps,
        core_ids=list(range(N_CORES)),
        trace=trace,
        trace_cores=trace_cores,
    )
    out = np.concatenate([r["out"] for r in res.results], axis=0)
    return out.reshape(B, S, D_OUT).astype(np.float32), res


def kernel(**inputs) -> np.ndarray:
    out, _ = run(inputs, trace=False)
    return out



# revision 7
# speedup vs baseline: 1.3327x; 1.3327x over previous
"""Trainium2 Bass kernel for ClassLinearWithLORA (moe_routing).

Computes out = x @ W.T + b + gates[-1] * (alpha * (x @ A[-1]) @ B_lora[-1])
(the torch loop overwrites out_lora each class iteration, so only the last
class adapter contributes).

Strategy (v2 — fp8 DoubleRow):
  - Data-parallel shard of the 8192 (B*S) rows across 8 NeuronCores
    (1024 rows/core); W/b and the rank-16 LoRA stacks are replicated.
  - All matmuls run in fp8e4 (e4m3) with MatmulPerfMode.DoubleRow: one
    instruction contracts 2x128 K-values at 0.5 cycles/row — 4x the
    fp32r contraction rate per instruction.
  - Precision is recovered with a compensated split: x = x_h + x_l and
    W' = 64*W = W_h + W_l (both parts fp8). The main GEMM runs 3 passes
    (x_h@W_h + x_h@W_l + x_l@W_h); the dropped x_l@W_l term is O(eps^2).
    Per output tile that is 12 DoubleRow matmuls (3072 cycles) vs 8
    fp32r matmuls (4096 cycles).
  - The LoRA first matmul (x@A) uses the same 3-pass split, with A's
    columns duplicated to 32 so the psum rows 16-31 carry a copy used
    to build the l_l part lane-aligned on the Vector engine.
  - The rank-16 LoRA update + bias fold into ONE DoubleRow aug matmul
    per tile (K = 2x32): subtile0 = [l_h; l_l] x [B_h; B_h], subtile1 =
    [l_h; ones; ones; 0...] x [B_l; b_h; b_l; 0...], where
    l = 16*g*(x@A) is built on-device in fp8 h+l parts.
  - Everything accumulates in one PSUM bank at scale S=64; the epilogue
    is a tensor_scalar_mul by 1/64 on the Vector engine writing fp16
    tiles, stored per-tile on the ACT HWDGE ring (last tile split
    across both rings to shorten the tail).

Measured: relative error ~2.2e-3 vs the fp32 jax reference (harness
gate 2e-2); per-core cost-model PE floor for this decomposition is
~91 us vs ~126 us for the fp32r formulation.
"""

import numpy as np
import ml_dtypes

import concourse.bacc as bacc
import concourse.mybir as mybir
import concourse.tile as tile
from concourse.bass_utils import run_bass_kernel_spmd

F32 = mybir.dt.float32
F16 = mybir.dt.float16
FP8 = mybir.dt.float8e4
DR = mybir.MatmulPerfMode.DoubleRow
NP8 = ml_dtypes.float8_e4m3

N_CORES = 8
B, S, D_IN, D_OUT = 4, 2048, 1024, 4096
R_LORA = 16
ROWS = B * S                  # 8192
R_CORE = ROWS // N_CORES      # 1024 rows per core
KT2 = D_IN // 256             # 4 double-K chunks (each 2x128)
NB = 512                      # moving free dim per matmul
OB = D_OUT // NB              # 8 output blocks
RT = R_CORE // 128            # 8 row tiles per core
SCALE = 64.0                  # PSUM carries 64x the true output


def _build():
    nc = bacc.Bacc(None, target_bir_lowering=False)

    xh_d = nc.dram_tensor("xh", [128, KT2, 2, R_CORE], FP8, kind="ExternalInput")
    xl_d = nc.dram_tensor("xl", [128, KT2, 2, R_CORE], FP8, kind="ExternalInput")
    wh_d = nc.dram_tensor("wh", [128, OB, KT2, 2, NB], FP8, kind="ExternalInput")
    wl_d = nc.dram_tensor("wl", [128, OB, KT2, 2, NB], FP8, kind="ExternalInput")
    ah_d = nc.dram_tensor("ah", [128, KT2, 2, 32], FP8, kind="ExternalInput")
    al_d = nc.dram_tensor("al", [128, KT2, 2, 32], FP8, kind="ExternalInput")
    rhs_d = nc.dram_tensor("rhs_aug", [32, 2, D_OUT], FP8, kind="ExternalInput")
    g_d = nc.dram_tensor("g2", [32, R_CORE], F32, kind="ExternalInput")
    msk_d = nc.dram_tensor("mask", [32, R_CORE], FP8, kind="ExternalInput")
    one_d = nc.dram_tensor("ones2", [2, R_CORE], FP8, kind="ExternalInput")
    zr_d = nc.dram_tensor("zeros14", [14, R_CORE], FP8, kind="ExternalInput")
    out_d = nc.dram_tensor("out", [R_CORE, D_OUT], F16, kind="ExternalOutput")

    with tile.TileContext(nc) as tc:
        with (
            tc.tile_pool(name="resident", bufs=1) as res,
            tc.tile_pool(name="wpool", bufs=5) as wpool,
            tc.tile_pool(name="opool", bufs=4) as opool,
            tc.tile_pool(name="psum", bufs=8, space="PSUM") as psum,
        ):
            # ---- resident loads -------------------------------------------
            # SP ring: A stacks + wt block 0 (chunked so the first matmuls
            # unblock early) + aug constants. ACT ring: x chunks (later the
            # output stores).
            ah = res.tile([128, KT2, 2, 32], FP8)
            nc.sync.dma_start(ah[:], ah_d.ap())
            al = res.tile([128, KT2, 2, 32], FP8)
            nc.sync.dma_start(al[:], al_d.ap())
            wh0 = wpool.tile([128, KT2, 2, NB], FP8, tag="wt", name="wh0")
            for t in range(KT2):
                nc.sync.dma_start(wh0[:, t], wh_d.ap()[:, 0, t])
            wl0 = wpool.tile([128, KT2, 2, NB], FP8, tag="wt", name="wl0")
            for t in range(KT2):
                nc.sync.dma_start(wl0[:, t], wl_d.ap()[:, 0, t])
            g2 = res.tile([32, R_CORE], F32)
            nc.sync.dma_start(g2[:], g_d.ap())
            msk = res.tile([32, R_CORE], FP8)
            nc.sync.dma_start(msk[:], msk_d.ap())
            rhs_aug = res.tile([32, 2, D_OUT], FP8)
            nc.sync.dma_start(rhs_aug[:], rhs_d.ap())
            laug = res.tile([32, 2, R_CORE], FP8)
            nc.sync.dma_start(laug[16:18, 1], one_d.ap())
            nc.sync.dma_start(laug[18:32, 1], zr_d.ap())
            v_sb = res.tile([32, R_CORE], F32)
            tmph = res.tile([32, R_CORE], FP8)
            m_sb = res.tile([32, R_CORE], FP8)

            xh = res.tile([128, KT2, 2, R_CORE], FP8)
            for t in range(KT2):
                nc.scalar.dma_start(xh[:, t], xh_d.ap()[:, t])
            xl = res.tile([128, KT2, 2, R_CORE], FP8)
            for t in range(KT2):
                nc.scalar.dma_start(xl[:, t], xl_d.ap()[:, t])

            # ---- prologue: ob=0 mains + LoRA, paced by chunk arrivals -----
            ps_l = [psum.tile([32, NB], F32, tag="ps", name=f"psl{rb}") for rb in range(2)]
            ps0 = [psum.tile([128, NB], F32, tag="ps", name=f"ps0_{rt}") for rt in range(6)]

            def lora_mm(xt_sb, a_sb, t, first, last):
                for rb in range(2):
                    nc.tensor.matmul(
                        ps_l[rb][:],
                        a_sb[:, t],
                        xt_sb[:, t, :, rb * NB : (rb + 1) * NB],
                        start=first, stop=last, perf_mode=DR,
                    )

            def main_mm(ps, xt_sb, wt_sb, t, rt, first, last=False):
                nc.tensor.matmul(
                    ps[:],
                    xt_sb[:, t, :, rt * 128 : (rt + 1) * 128],
                    wt_sb[:, t],
                    start=first, stop=last, perf_mode=DR,
                )

            stages = ((xh, wh0, ah), (xh, wl0, al), (xl, wh0, ah))
            for si, (xt_sb, wt_sb, a_sb) in enumerate(stages):
                for t in range(KT2):
                    lora_mm(xt_sb, a_sb, t,
                            first=(si == 0 and t == 0),
                            last=(si == 2 and t == KT2 - 1))
                    for rt in range(6):
                        main_mm(ps0[rt], xt_sb, wt_sb, t, rt,
                                first=(si == 0 and t == 0))

            # gated LoRA intermediate -> fp8 h+l parts. Engine SBUF accesses
            # must start at a 32-aligned partition, so sub0 = [l_h; l_l] is
            # produced by full 32-lane ops: psum rows 16-31 duplicate rows
            # 0-15 (duplicated A columns); mask is 0 on lanes 0-15 and 1 on
            # lanes 16-31, so fp8(v - fp8(v)*mask) = [l_h; l_l] in one op.
            for rb in range(2):
                sl = slice(rb * NB, (rb + 1) * NB)
                nc.vector.tensor_mul(v_sb[:, sl], ps_l[rb][:], g2[:, sl])
            nc.vector.tensor_copy(tmph[:], v_sb[:])
            nc.vector.tensor_mul(m_sb[:], tmph[:], msk[:])
            nc.vector.tensor_sub(laug[:, 0], v_sb[:], m_sb[:])
            nc.vector.tensor_copy(laug[0:16, 1], v_sb[0:16])

            # rt 6,7 mains keep the PE busy while the DVE builds laug
            ps67 = {}
            for rt in (6, 7):
                ps = psum.tile([128, NB], F32, tag="ps", name=f"ps0b{rt}")
                first = True
                for xt_sb, wt_sb, _ in stages:
                    for t in range(KT2):
                        main_mm(ps, xt_sb, wt_sb, t, rt, first=first)
                        first = False
                ps67[rt] = ps

            def emit_aug(ps, rt, ob, start, stop):
                nc.tensor.matmul(
                    ps[:],
                    laug[:, :, rt * 128 : (rt + 1) * 128],
                    rhs_aug[:, :, ob * NB : (ob + 1) * NB],
                    start=start, stop=stop, perf_mode=DR,
                )

            def emit_epilogue(ps, rt, ob, last=False):
                o_sb = opool.tile([128, NB], F16, tag="o", name=f"o{ob}_{rt}")
                orow = out_d.ap()[rt * 128 : (rt + 1) * 128, ob * NB : (ob + 1) * NB]
                if last:
                    h = NB // 2
                    nc.vector.tensor_scalar_mul(o_sb[:, 0:h], ps[:, 0:h], 1.0 / SCALE)
                    nc.scalar.dma_start(orow[:, 0:h], o_sb[:, 0:h])
                    nc.vector.tensor_scalar_mul(o_sb[:, h:NB], ps[:, h:NB], 1.0 / SCALE)
                    nc.sync.dma_start(orow[:, h:NB], o_sb[:, h:NB])
                else:
                    nc.vector.tensor_scalar_mul(o_sb[:], ps[:], 1.0 / SCALE)
                    nc.scalar.dma_start(orow[:], o_sb[:])

            for rt in range(6):
                emit_aug(ps0[rt], rt, 0, start=False, stop=True)
                emit_epilogue(ps0[rt], rt, 0)
            for rt in (6, 7):
                emit_aug(ps67[rt], rt, 0, start=False, stop=True)
                emit_epilogue(ps67[rt], rt, 0)

            # ---- steady state: ob = 1..7 ----------------------------------
            for ob in range(1, OB):
                whb = wpool.tile([128, KT2, 2, NB], FP8, tag="wt", name=f"wh{ob}")
                for t in range(KT2):
                    nc.sync.dma_start(whb[:, t], wh_d.ap()[:, ob, t])
                wlb = wpool.tile([128, KT2, 2, NB], FP8, tag="wt", name=f"wl{ob}")
                for t in range(KT2):
                    nc.sync.dma_start(wlb[:, t], wl_d.ap()[:, ob, t])
                for rt in range(RT):
                    ps = psum.tile([128, NB], F32, tag="ps", name=f"ps{ob}_{rt}")
                    emit_aug(ps, rt, ob, start=True, stop=False)
                    for si, (xt_sb, wt_sb) in enumerate(
                        ((xh, whb), (xh, wlb), (xl, whb))
                    ):
                        for t in range(KT2):
                            main_mm(ps, xt_sb, wt_sb, t, rt, first=False,
                                    last=(si == 2 and t == KT2 - 1))
                    emit_epilogue(ps, rt, ob, last=(ob == OB - 1 and rt == RT - 1))

    nc.compile()
    return nc


_NC_CACHE = None


def _get_nc():
    global _NC_CACHE
    if _NC_CACHE is None:
        _NC_CACHE = _build()
    return _NC_CACHE


def _hi_lo(a):
    """fp32 array -> (high fp8, low fp8) with a + err = high + low + O(eps^2)."""
    h = np.ascontiguousarray(a, dtype=np.float32).astype(NP8)
    l = (a - h.astype(np.float32)).astype(NP8)
    return h, l


def _prep_in_maps(x, W, b, A, B_lora, gates, alpha):
    x = np.asarray(x, dtype=np.float32).reshape(ROWS, D_IN)
    W = np.asarray(W, dtype=np.float32)
    b = np.asarray(b, dtype=np.float32)
    A1 = np.asarray(A, dtype=np.float32)[-1]          # [D_IN, 16]
    B1 = np.asarray(B_lora, dtype=np.float32)[-1]     # [16, D_OUT]
    g = np.asarray(gates, dtype=np.float32)[-1].reshape(ROWS)
    alpha_f = float(np.asarray(alpha))

    # W' = 64*W packed [ki, ob, t, i, n] with k = t*256 + i*128 + ki
    wh, wl = _hi_lo((SCALE * W).astype(np.float32).T)          # [K, O]

    def pack_w(wq):
        return np.ascontiguousarray(
            wq.reshape(KT2, 2, 128, OB, NB).transpose(2, 3, 0, 1, 4))

    wh_p, wl_p = pack_w(wh), pack_w(wl)

    # A' = 64*A with columns duplicated to 32 (psum rows 16-31 = copy)
    ahq, alq = _hi_lo((SCALE * A1).astype(np.float32))         # [K, 16]

    def pack_a(aq):
        a32 = np.concatenate([aq, aq], axis=1)                 # [K, 32]
        return np.ascontiguousarray(
            a32.reshape(KT2, 2, 128, 32).transpose(2, 0, 1, 3))

    ah_p, al_p = pack_a(ahq), pack_a(alq)

    # aug rhs: sub0 = [B_h; B_h], sub1 = [B_l; b_h; b_l; 0...]
    # B' = 4*alpha*B so that (16*g*xA) @ B' = 64 * g*alpha*(xA@B)
    bh8, bl8 = _hi_lo((SCALE / 16.0 * alpha_f * B1).astype(np.float32))
    bbh, bbl = _hi_lo((SCALE * b).astype(np.float32))
    rhs = np.zeros((32, 2, D_OUT), dtype=NP8)
    rhs[0:16, 0] = bh8
    rhs[16:32, 0] = bh8
    rhs[0:16, 1] = bl8
    rhs[16, 1] = bbh
    rhs[17, 1] = bbl

    ones2 = np.ones((2, R_CORE), dtype=NP8)
    zr = np.zeros((14, R_CORE), dtype=NP8)
    mask = np.zeros((32, R_CORE), dtype=NP8)
    mask[16:32] = 1.0

    def pack_x(xq):
        return np.ascontiguousarray(
            xq.reshape(KT2, 2, 128, R_CORE).transpose(2, 0, 1, 3))

    in_maps = []
    for c in range(N_CORES):
        rows = slice(c * R_CORE, (c + 1) * R_CORE)
        xhq, xlq = _hi_lo(x[rows].T)                           # [K, R_CORE]
        g2 = np.ascontiguousarray(
            np.broadcast_to((0.25 * g[rows])[None, :], (32, R_CORE))
        ).astype(np.float32)
        in_maps.append(
            {
                "xh": pack_x(xhq),
                "xl": pack_x(xlq),
                "wh": wh_p,
                "wl": wl_p,
                "ah": ah_p,
                "al": al_p,
                "rhs_aug": rhs,
                "g2": g2,
                "mask": mask,
                "ones2": ones2,
                "zeros14": zr,
            }
        )
    return in_maps


def run(inputs: dict, trace: bool = False, trace_cores=None):
    """Run the kernel; returns (full_output, BassKernelResults)."""
    nc = _get_nc()
    in_maps = _prep_in_maps(**inputs)
    res = run_bass_kernel_spmd(
        nc,
        in_maps,
        core_ids=list(range(N_CORES)),
        trace=trace,
        trace_cores=trace_cores,
    )
    out = np.concatenate(
        [np.asarray(r["out"]).astype(np.float32) for r in res.results], axis=0
    )
    return out.reshape(B, S, D_OUT), res


def kernel(**inputs) -> np.ndarray:
    out, _ = run(inputs, trace=False)
    return out
